# revision 46
# baseline (speedup 1.0000x reference)
"""Trainium2 Bass kernel for nn_Attention_18399639896530.

Reference computation (b=2, c=256, l=4096, heads=4, dim_head=32):
  qkv   = w_qkv @ x[b]                  (pointwise conv == channel matmul)
  q,k,v -> (b, h, d, l);  q,k L2-normalized over the *sequence* axis l
  sim   = 10 * q^T k    (per b,h: (l, l));  attn = softmax(sim, -1)
  out   = attn @ v^T -> (b, h, l, d);  y = w_out @ scrambled-reshape + b_out

Key numerical fact: because q,k are normalized along the SEQUENCE axis,
|sim| <= ~0.11 on these inputs, so exp(sim) = 1 + sim to 1.4e-4 relative
accuracy (the gate is 2e-2).  The softmax therefore collapses to LINEAR
attention computed through two tiny matrices:

  kT1 = [K^T | 1]  (4096 x 33),  vT1 = [V^T | 1]
  M'  = kT1^T vT1                       (33 x 33; row 32 = [sum_j v_j | L])
  T   = X^T (Wq^T diag(10 rq rk) M'[0:32]) + 1 * M'[32]     (L x 33)
        --- per-column i: T[i, 0:32] = sum_j e_ij v_j,  T[i,32] = Z_i
  O   = T[:, 0:32] / T[:, 32]  ->  scrambled reshape -> y = wo^T.T @ R

The scrambled reshape needs R[r', (u, dd)] with r' = i//128 on partitions.
Instead of partition-flattening DMAs (per-partition SBUF bandwidth bound,
~6us each), R is built with two rounds of PE transposes:
  TZ [dd, i]  --32x [32,128] transposes-->  OTn [u, jb, dd]
  OTn[:, :, dd]  --32x [128,32-strided] transposes-->  R [r', dd, u]
P1 computes kvT directly (x blocks stationary) so kT/vT/qT land in
transposed layout straight from the projection matmul - no kvN staging.
Sharding: 8 cores == 8 (b, h) pairs; host sums the 4 per-head partials per
batch and adds b_out.
"""

import os
import sys
import numpy as np

try:
    import concourse  # noqa: F401
except ImportError:  # pragma: no cover
    sys.path.insert(0, "/opt/trn_rl_repo")

import concourse.bass as bass  # noqa: E402
import concourse.tile as tile  # noqa: E402
from concourse import bacc, mybir  # noqa: E402
from concourse import bass_utils  # noqa: E402
from concourse.masks import make_identity  # noqa: E402

B, C, L = 2, 256, 4096
H, D = 4, 32
NJ = L // 128       # 32 j-blocks
F32 = mybir.dt.float32
F32R = mybir.dt.float32r
BF16 = mybir.dt.bfloat16

_CACHE = {}
DEBUG_DUMP = False


def _act_recip(nc, out, in_, bias):
    """out = 1/(in_ + bias) on the Activation engine (single-pass table
    op).  bass's activation() refuses Reciprocal wholesale; the achievable
    accuracy (~1e-5 relative here, denominators ~4096) is far inside this
    kernel's 2e-2 budget."""
    imm = lambda v: mybir.ImmediateValue(dtype=mybir.dt.float32, value=v)
    return nc.scalar.add_instruction(
        mybir.InstActivation(
            name=nc.get_next_instruction_name(),
            func=mybir.ActivationFunctionType.Reciprocal,
            ins=[nc.scalar.lower_ap(in_), imm(bias), imm(1.0), imm(0.0)],
            outs=[nc.scalar.lower_ap(out)],
        ))


def _setup(tc, P):
    """Compile-time constants, emitted once before the repeat loop."""
    nc = tc.nc
    cst = P["cst"]
    identF = cst.tile([128, 128], F32)
    make_identity(nc, identF)
    identB = cst.tile([128, 128], BF16)
    nc.vector.tensor_copy(identB, identF)
    P["identF"] = identF
    P["identB"] = identB


def _emit(tc, P, y_d, x_d, wkvm_d, wqg_d, wob_d, upto=99, xbar=False):
    nc = tc.nc
    ping, work = P["ping"], P["work"]
    psKV, psS, psMG = P["psKV"], P["psS"], P["psMG"]
    identF, identB = P["identF"], P["identB"]
    ident = identF[0:D + 1, 0:D + 1]

    # ---- load inputs (small weights first) ---------------------------
    # wkq cols (= kvN rows): [wk(0:32) | 0(32) | 0(33:64) | wq(64:96)
    #                         | wv(96:128)]
    wkq_sb = ping.tile([128, 2, 128], BF16, tag="wkq")
    nc.sync.dma_start(wkq_sb, wkvm_d)
    w2_sb = ping.tile([D, 2, 128], F32R, tag="w2")        # [a, cc, wq rows]
    nc.gpsimd.dma_start(w2_sb, wqg_d)
    wob_sb = ping.tile([D, 2, 128], BF16, tag="wob")      # [r', mc, wo rows]
    nc.gpsimd.dma_start(wob_sb, wob_d)
    x_sb = ping.tile([128, 2, L], BF16, tag="x")          # [c%128, c//128, l]
    xr = x_d.rearrange("(cc p) l -> p cc l", p=128)
    for lq in range(4):
        (nc.sync if lq % 2 == 0 else nc.gpsimd).dma_start(
            x_sb[:, :, lq * 1024:(lq + 1) * 1024],
            xr[:, :, lq * 1024:(lq + 1) * 1024])

    # ---- P1: kvN = wkqv^T X  (16 wide matmuls), then xbar -> kvT -----
    # kvN rows: [kT(0:32) | 0(32) | 0(33:64) | qT(64:96) | vT(96:128)];
    # row 32 becomes the kvT ones-col via a post-xbar memset.
    kvN_sb = ping.tile([128, L], BF16, tag="kvN")
    kvT_sb = ping.tile([128, NJ, 128], BF16, tag="kvT")
    cp_eng = [nc.vector.tensor_copy, nc.scalar.copy]
    for lq in range(8):
        kvn_ps = psKV.tile([128, 512], F32, tag="kvt")
        for cc in range(2):
            nc.tensor.matmul(kvn_ps, wkq_sb[:, cc, :],
                             x_sb[:, cc, lq * 512:(lq + 1) * 512],
                             start=(cc == 0), stop=(cc == 1),
                             skip_group_check=True)
        cp_eng[lq % 2](kvN_sb[:, lq * 512:(lq + 1) * 512], kvn_ps)
    nc.sync.dma_start_transpose(kvT_sb[:], kvN_sb[:])
    nc.gpsimd.memset(kvT_sb[:, :, 32:33], 1.0)

    # ---- single wide gram over all of [kT|1|0|qT|vT] -----------------
    #   GG[0:33, 0:33]  = kT1 gram (k norms; col 32 = [sum k | L])
    #   GG[64:96, 64:96] diag = q norms;  GG[0:33, 96:128] = kT1^T vT
    GG_ps = psMG.tile([96, 128], F32, tag="mg")
    for jb in range(NJ):
        nc.tensor.matmul(GG_ps, kvT_sb[:, jb, 0:96], kvT_sb[:, jb, :],
                         start=(jb == 0), stop=(jb == NJ - 1),
                         skip_group_check=True)
    if upto <= 1:
        return

    # ---- fold both norms + SCALE into 10/(||q_a|| ||k_a||) -----------
    gd = ping.tile([D + 1, D + 1], F32, tag="gd")
    nc.vector.tensor_mul(gd, GG_ps[0:33, 0:33], ident)
    nks = ping.tile([D + 1, 1], F32, tag="nks")
    nc.vector.tensor_reduce(nks, gd, axis=mybir.AxisListType.X,
                            op=mybir.AluOpType.add)
    # q norms sit at partitions 64:96; a tiny fp32 matmul with a shifted
    # identity (nonzeros at (64+a, a)) moves them down to 0:32.
    nqs128 = ping.tile([128, 1], F32, tag="nqs")
    nc.gpsimd.memset(nqs128, 0.0)
    gd2 = ping.tile([128, D], F32, tag="gd2")
    nc.vector.tensor_mul(gd2[64:96, :], GG_ps[64:96, 64:96],
                         identF[64:96, 64:96])
    nc.vector.tensor_reduce(nqs128[64:96, :], gd2[64:96, :],
                            axis=mybir.AxisListType.X,
                            op=mybir.AluOpType.add)
    nq_ps = psMG.tile([D, 1], F32, tag="qg")
    nc.tensor.matmul(nq_ps, identF[:, 64:96], nqs128,
                     start=True, stop=True)
    m = ping.tile([D, 1], F32, tag="m")
    nc.vector.tensor_mul(m, nq_ps, nks[0:32])
    sqm = ping.tile([D, 1], F32, tag="sqm")
    nc.scalar.activation(sqm, m, mybir.ActivationFunctionType.Sqrt)
    f10 = ping.tile([D, 1], F32, tag="f10")
    nc.vector.reciprocal(f10, sqm)

    # ---- Msb = diag([f | 1]) M'raw, Z-col replicated to 33:64 --------
    # (x10 folded into Gsb).  T rows 32:64 all carry Z so the epilogue
    # reciprocal+mul are full-width DVE ops (no partition broadcast).
    # Row 32 (-> M32c): [sum_j v | junk]; col 32 = f * [sum_j k].
    Msb = ping.tile([D + 1, 2 * D], F32R, tag="Msb")
    nc.vector.tensor_scalar_mul(Msb[0:32, 0:32], GG_ps[0:32, 96:128], f10)
    nc.vector.tensor_scalar_mul(Msb[0:32, 32:33], GG_ps[0:32, 32:33], f10)
    nc.scalar.copy(Msb[32:33, 0:32], GG_ps[32:33, 96:128])
    nc.gpsimd.memset(Msb[32:33, 32:33].bitcast(F32), float(L))
    w = 1
    while 32 + w < 2 * D:
        cw = min(w, 2 * D - 32 - w)
        nc.vector.tensor_copy(Msb[:, 32 + w:32 + w + cw],
                              Msb[:, 32:32 + cw])
        w += cw
    m32t_ps = psMG.tile([D + 1, 1], F32, tag="mg")
    nc.tensor.transpose(m32t_ps, Msb[32:33, 0:33].bitcast(F32),
                        identF[32:33, 32:33])
    M32c = ping.tile([D + 1, 1], F32, tag="M32c")
    nc.vector.tensor_copy(M32c, m32t_ps)
    G_ps = psMG.tile([128, 2, 2 * D], F32, tag="mg")
    for cc in range(2):
        nc.tensor.matmul(G_ps[:, cc, :], w2_sb[:, cc, :],
                         Msb[0:32, :], start=True, stop=True)
    Gsb = ping.tile([128, 2, 2 * D], BF16, tag="Gsb")
    nc.vector.tensor_scalar_mul(Gsb, G_ps, 10.0)
    if upto <= 2:
        return

    # ---- T = X^T G + ones*M'[32]; normalize; transpose to OTn --------
    # OTn[u, jb, dd] = O[dd, jb*128+u]  (bf16)
    OTn_sb = ping.tile([128, NJ, D], BF16, tag="OTn")
    TZfull = (ping.tile([D, L], BF16, tag="TZf", name="TZfull")
              if xbar else None)
    for tq in range(8):
        T_ps = psS.tile([2 * D, 512], F32, tag="s")
        for cc in range(2):
            nc.tensor.matmul(T_ps, Gsb[:, cc, :],
                             x_sb[:, cc, tq * 512:(tq + 1) * 512],
                             start=(cc == 0), stop=(cc == 1))
        rzt = work.tile([D, 512], F32, tag="rz", bufs=3)
        _act_recip(nc, rzt, T_ps[32:64, :], float(L))
        if xbar:
            TZ = TZfull[:, tq * 512:(tq + 1) * 512]
        else:
            TZ = work.tile([D, 512], BF16, tag="TZ", bufs=3)
        nc.vector.scalar_tensor_tensor(TZ, T_ps[0:32, :], M32c[0:32],
                                       rzt, mybir.AluOpType.add,
                                       mybir.AluOpType.mult)
        if upto <= 3:
            continue
        if not xbar:
            otn_ps = psKV.tile([128, 4, D], F32, tag="kvt")
            for r in range(4):
                nc.tensor.matmul(otn_ps[:, r, :],
                                 TZ[:, r * 128:(r + 1) * 128],
                                 identB[0:32, 0:32], start=True, stop=True)
            cp_eng[tq % 2](OTn_sb[:, 4 * tq:4 * tq + 4, :], otn_ps)
    if xbar and upto > 3:
        nc.sync.dma_start_transpose(OTn_sb[:], TZfull[:])
    if upto <= 4:
        return
    if DEBUG_DUMP:
        dbg = P["dbg_d"]
        nc.sync.dma_start(dbg[0:32, :], TZfull)
        nc.gpsimd.dma_start(dbg[32:160, 0:NJ * D],
                            OTn_sb.rearrange("p j c -> p (j c)"))

    # ---- R[r', dd, u] via per-dd transposes of OTn -------------------
    R_sb = ping.tile([D, D, 128], BF16, tag="R")
    for ddq in range(8):
        r4_ps = psKV.tile([D, 4, 128], F32, tag="kvt")
        for s in range(4):
            dd = 4 * ddq + s
            nc.tensor.matmul(r4_ps[:, s, :], OTn_sb[:, :, dd],
                             identB, start=True, stop=True)
        cp_eng[ddq % 2](R_sb[:, 4 * ddq:4 * ddq + 4, :], r4_ps)
    if upto <= 5:
        return
    if DEBUG_DUMP:
        nc.sync.dma_start(P["dbg_d"][160:192, :],
                          R_sb.rearrange("p d u -> p (d u)"))

    # ---- final projection: y = wo^T.T @ R ----------------------------
    i = 0
    for mc in range(2):
        for ng in range(2):
            y_sb = work.tile([128, 4, 512], BF16, tag="ysb", bufs=2)
            for sub in range(4):
                ncq = ng * 4 + sub
                y_ps = psS.tile([128, 512], F32, tag="s")
                nc.tensor.matmul(
                    y_ps, wob_sb[:, mc, :],
                    R_sb[:, :, ncq * 16:(ncq + 1) * 16].rearrange(
                        "r d u -> r u d"),
                    start=True, stop=True)
                cp_eng[i % 2](y_sb[:, sub, :], y_ps)
                i += 1
            (nc.sync if (mc + ng) % 2 == 0 else nc.gpsimd).dma_start(
                y_d[mc * 128:(mc + 1) * 128,
                    ng * 2048:(ng + 1) * 2048], y_sb)


def _build_program(repeat=1, upto=99, xbar=False):
    key = ("nc", repeat, upto, xbar)
    names_key = ("names", repeat, upto, xbar)
    if key in _CACHE:
        return _CACHE[key], _CACHE[names_key]
    nc = bacc.Bacc("TRN2", target_bir_lowering=False, debug=False,
                   enable_asserts=False, num_devices=8)
    x_d = nc.dram_tensor("x", (C, L), BF16, kind="ExternalInput").ap()
    wkvm_d = nc.dram_tensor("wkvm", (128, 2, 128), BF16,
                            kind="ExternalInput").ap()
    wqg_d = nc.dram_tensor("wqg", (D, 2, 128), F32R,
                           kind="ExternalInput").ap()
    wob_d = nc.dram_tensor("wob", (D, 2, 128), BF16,
                           kind="ExternalInput").ap()
    y_d = nc.dram_tensor("y", (C, L), BF16, kind="ExternalOutput").ap()
    dbg_d = (nc.dram_tensor("dbg", (192, L), BF16,
                            kind="ExternalOutput").ap()
             if DEBUG_DUMP else None)
    from contextlib import ExitStack
    with tile.TileContext(nc) as tc, ExitStack() as ctx:
        P = {
            "cst": ctx.enter_context(tc.tile_pool(name="cst", bufs=1)),
            "ping": ctx.enter_context(tc.tile_pool(name="ping", bufs=2)),
            "work": ctx.enter_context(tc.tile_pool(name="work", bufs=2)),
            "psKV": ctx.enter_context(
                tc.tile_pool(name="psKV", bufs=2, space="PSUM")),
            "psS": ctx.enter_context(
                tc.tile_pool(name="psS", bufs=3, space="PSUM")),
            "psMG": ctx.enter_context(
                tc.tile_pool(name="psMG", bufs=1, space="PSUM")),
        }
        P["dbg_d"] = dbg_d
        _setup(tc, P)
        if repeat == 1:
            _emit(tc, P, y_d, x_d, wkvm_d, wqg_d, wob_d, upto, xbar)
        else:
            with tc.For_i(0, repeat // 2, 1):
                _emit(tc, P, y_d, x_d, wkvm_d, wqg_d, wob_d, upto, xbar)
                _emit(tc, P, y_d, x_d, wkvm_d, wqg_d, wob_d, upto, xbar)
    nc.compile()
    names = dict(x=x_d.name, wkvm=wkvm_d.name, wqg=wqg_d.name,
                 wob=wob_d.name, y=y_d.name)
    _CACHE[key] = nc
    _CACHE[names_key] = names
    return nc, names


def _in_maps(x, w_qkv, w_out, names):
    import ml_dtypes
    maps = []
    for core in range(8):
        b, h = divmod(core, H)
        wq = w_qkv[h * D:(h + 1) * D]                  # [32, 256]
        wk = w_qkv[128 + h * D:128 + (h + 1) * D]
        wv = w_qkv[256 + h * D:256 + (h + 1) * D]
        wkvq = np.zeros((128, 256), np.float32)
        wkvq[0:32], wkvq[64:96], wkvq[96:128] = wk, wq, wv
        wkvm = np.ascontiguousarray(
            wkvq.T.reshape(2, 128, 128).transpose(1, 0, 2))
        wqg = np.ascontiguousarray(wq.reshape(D, 2, 128))
        wob = np.ascontiguousarray(
            w_out[:, h * D:(h + 1) * D].T.reshape(D, 2, 128))
        maps.append({
            names["x"]: np.ascontiguousarray(x[b]).astype(ml_dtypes.bfloat16),
            names["wkvm"]: wkvm.astype(ml_dtypes.bfloat16),
            names["wqg"]: wqg,
            names["wob"]: wob.astype(ml_dtypes.bfloat16),
        })
    return maps


def run(x, w_qkv, w_out, b_out, **spmd_kwargs):
    """Build+run; returns (y_full, BassKernelResults)."""
    x = np.asarray(x, np.float32)
    w_qkv = np.asarray(w_qkv, np.float32)
    w_out = np.asarray(w_out, np.float32)
    b_out = np.asarray(b_out, np.float32)
    repeat = spmd_kwargs.pop("repeat", 1)
    upto = spmd_kwargs.pop("upto", 99)
    xbar = spmd_kwargs.pop("xbar", True)
    nc, names = _build_program(repeat, upto, xbar)
    res = bass_utils.run_bass_kernel_spmd(
        nc, _in_maps(x, w_qkv, w_out, names), core_ids=list(range(8)),
        **spmd_kwargs)
    y = np.zeros((B, C, L), np.float32)
    for core in range(8):
        y[core // H] += np.asarray(res.results[core][names["y"]],
                                   dtype=np.float32)
    y += b_out[None, :, None]
    return y, res


def kernel(x, w_qkv, w_out, b_out):
    y, _ = run(x, w_qkv, w_out, b_out)
    return y


# revision 48
# speedup vs baseline: 1.0240x; 1.0240x over previous
"""Trainium2 Bass kernel for nn_Attention_18399639896530.

Reference computation (b=2, c=256, l=4096, heads=4, dim_head=32):
  qkv   = w_qkv @ x[b]                  (pointwise conv == channel matmul)
  q,k,v -> (b, h, d, l);  q,k L2-normalized over the *sequence* axis l
  sim   = 10 * q^T k    (per b,h: (l, l));  attn = softmax(sim, -1)
  out   = attn @ v^T -> (b, h, l, d);  y = w_out @ scrambled-reshape + b_out

Key numerical fact: because q,k are normalized along the SEQUENCE axis,
|sim| <= ~0.11 on these inputs, so exp(sim) = 1 + sim to 1.4e-4 relative
accuracy (the gate is 2e-2).  The softmax therefore collapses to LINEAR
attention computed through two tiny matrices:

  kT1 = [K^T | 1]  (4096 x 33),  vT1 = [V^T | 1]
  M'  = kT1^T vT1                       (33 x 33; row 32 = [sum_j v_j | L])
  T   = X^T (Wq^T diag(10 rq rk) M'[0:32]) + 1 * M'[32]     (L x 33)
        --- per-column i: T[i, 0:32] = sum_j e_ij v_j,  T[i,32] = Z_i
  O   = T[:, 0:32] / T[:, 32]  ->  scrambled reshape -> y = wo^T.T @ R

The scrambled reshape needs R[r', (u, dd)] with r' = i//128 on partitions.
Instead of partition-flattening DMAs (per-partition SBUF bandwidth bound,
~6us each), R is built with two rounds of PE transposes:
  TZ [dd, i]  --32x [32,128] transposes-->  OTn [u, jb, dd]
  OTn[:, :, dd]  --32x [128,32-strided] transposes-->  R [r', dd, u]
P1 computes kvT directly (x blocks stationary) so kT/vT/qT land in
transposed layout straight from the projection matmul - no kvN staging.
Sharding: 8 cores == 8 (b, h) pairs; host sums the 4 per-head partials per
batch and adds b_out.
"""

import os
import sys
import numpy as np

try:
    import concourse  # noqa: F401
except ImportError:  # pragma: no cover
    sys.path.insert(0, "/opt/trn_rl_repo")

import concourse.bass as bass  # noqa: E402
import concourse.tile as tile  # noqa: E402
from concourse import bacc, mybir  # noqa: E402
from concourse import bass_utils  # noqa: E402
from concourse.masks import make_identity  # noqa: E402

B, C, L = 2, 256, 4096
H, D = 4, 32
NJ = L // 128       # 32 j-blocks
F32 = mybir.dt.float32
F32R = mybir.dt.float32r
BF16 = mybir.dt.bfloat16

_CACHE = {}
DEBUG_DUMP = False


def _act_recip(nc, out, in_, bias):
    """out = 1/(in_ + bias) on the Activation engine (single-pass table
    op).  bass's activation() refuses Reciprocal wholesale; the achievable
    accuracy (~1e-5 relative here, denominators ~4096) is far inside this
    kernel's 2e-2 budget."""
    imm = lambda v: mybir.ImmediateValue(dtype=mybir.dt.float32, value=v)
    return nc.scalar.add_instruction(
        mybir.InstActivation(
            name=nc.get_next_instruction_name(),
            func=mybir.ActivationFunctionType.Reciprocal,
            ins=[nc.scalar.lower_ap(in_), imm(bias), imm(1.0), imm(0.0)],
            outs=[nc.scalar.lower_ap(out)],
        ))


def _setup(tc, P):
    """Compile-time constants, emitted once before the repeat loop."""
    nc = tc.nc
    cst = P["cst"]
    identF = cst.tile([128, 128], F32)
    make_identity(nc, identF)
    identB = cst.tile([128, 128], BF16)
    nc.vector.tensor_copy(identB, identF)
    P["identF"] = identF
    P["identB"] = identB


def _emit(tc, P, y_d, x_d, wkvm_d, wqg_d, wob_d, upto=99, xbar=False):
    nc = tc.nc
    ping, work = P["ping"], P["work"]
    psKV, psS, psMG = P["psKV"], P["psS"], P["psMG"]
    identF, identB = P["identF"], P["identB"]
    ident = identF[0:D + 1, 0:D + 1]

    # ---- load inputs (small weights first) ---------------------------
    # wkq cols (= kvN rows): [wk(0:32) | 0(32) | 0(33:64) | wq(64:96)
    #                         | wv(96:128)]
    wkq_sb = ping.tile([128, 2, 128], BF16, tag="wkq")
    nc.sync.dma_start(wkq_sb, wkvm_d)
    w2_sb = ping.tile([D, 2, 128], F32R, tag="w2")        # [a, cc, wq rows]
    nc.gpsimd.dma_start(w2_sb, wqg_d)
    wob_sb = ping.tile([D, 2, 128], BF16, tag="wob")      # [r', mc, wo rows]
    nc.gpsimd.dma_start(wob_sb, wob_d)
    x_sb = ping.tile([128, 2, L], BF16, tag="x")          # [c%128, c//128, l]
    xr = x_d.rearrange("(cc p) l -> p cc l", p=128)
    for lq in range(4):
        (nc.sync if lq % 2 == 0 else nc.gpsimd).dma_start(
            x_sb[:, :, lq * 1024:(lq + 1) * 1024],
            xr[:, :, lq * 1024:(lq + 1) * 1024])

    # ---- P1: kvN = wkqv^T X  (16 wide matmuls), then xbar -> kvT -----
    # kvN rows: [kT(0:32) | 0(32) | 0(33:64) | qT(64:96) | vT(96:128)];
    # row 32 becomes the kvT ones-col via a post-xbar memset.
    kvN_sb = ping.tile([128, L], BF16, tag="kvN")
    kvT_sb = ping.tile([128, NJ, 128], BF16, tag="kvT")
    cp_eng = [nc.vector.tensor_copy, nc.scalar.copy]

    # single wide gram over all of [kT|1|0|qT|vT]:
    #   GG[0:33, 0:33]  = kT1 gram (k norms; col 32 = [sum k | L])
    #   GG[64:96, 64:96] diag = q norms;  GG[0:33, 96:128] = kT1^T vT
    # The kvN->kvT xbar is split into quarters so gram(q-1) overlaps
    # P1 matmuls and the xbar of quarter q.
    GG_ps = psMG.tile([96, 128], F32, tag="mg")

    def _gram(q):
        for jb in range(8 * q, 8 * q + 8):
            nc.tensor.matmul(GG_ps, kvT_sb[:, jb, 0:96], kvT_sb[:, jb, :],
                             start=(jb == 0), stop=(jb == NJ - 1),
                             skip_group_check=True)

    for lq in range(8):
        kvn_ps = psKV.tile([128, 512], F32, tag="kvt")
        for cc in range(2):
            nc.tensor.matmul(kvn_ps, wkq_sb[:, cc, :],
                             x_sb[:, cc, lq * 512:(lq + 1) * 512],
                             start=(cc == 0), stop=(cc == 1),
                             skip_group_check=True)
        cp_eng[lq % 2](kvN_sb[:, lq * 512:(lq + 1) * 512], kvn_ps)
        if lq % 2 == 1:
            q = lq // 2
            nc.sync.dma_start_transpose(
                kvT_sb[:, 8 * q:8 * q + 8, :],
                kvN_sb[:, q * 1024:(q + 1) * 1024])
            nc.gpsimd.memset(kvT_sb[:, 8 * q:8 * q + 8, 32:33], 1.0)
            if q > 0:
                _gram(q - 1)
    _gram(3)
    if upto <= 1:
        return

    # ---- fold both norms + SCALE into 10/(||q_a|| ||k_a||) -----------
    gd = ping.tile([D + 1, D + 1], F32, tag="gd")
    nc.vector.tensor_mul(gd, GG_ps[0:33, 0:33], ident)
    nks = ping.tile([D + 1, 1], F32, tag="nks")
    nc.vector.tensor_reduce(nks, gd, axis=mybir.AxisListType.X,
                            op=mybir.AluOpType.add)
    # q norms sit at partitions 64:96; a tiny fp32 matmul with a shifted
    # identity (nonzeros at (64+a, a)) moves them down to 0:32.
    nqs128 = ping.tile([128, 1], F32, tag="nqs")
    nc.gpsimd.memset(nqs128, 0.0)
    gd2 = ping.tile([128, D], F32, tag="gd2")
    nc.vector.tensor_mul(gd2[64:96, :], GG_ps[64:96, 64:96],
                         identF[64:96, 64:96])
    nc.vector.tensor_reduce(nqs128[64:96, :], gd2[64:96, :],
                            axis=mybir.AxisListType.X,
                            op=mybir.AluOpType.add)
    nq_ps = psMG.tile([D, 1], F32, tag="qg")
    nc.tensor.matmul(nq_ps, identF[:, 64:96], nqs128,
                     start=True, stop=True)
    m = ping.tile([D, 1], F32, tag="m")
    nc.vector.tensor_mul(m, nq_ps, nks[0:32])
    sqm = ping.tile([D, 1], F32, tag="sqm")
    nc.scalar.activation(sqm, m, mybir.ActivationFunctionType.Sqrt)
    f10 = ping.tile([D, 1], F32, tag="f10")
    nc.vector.reciprocal(f10, sqm)

    # ---- Msb = diag([f | 1]) M'raw, Z-col replicated to 33:64 --------
    # (x10 folded into Gsb).  T rows 32:64 all carry Z so the epilogue
    # reciprocal+mul are full-width DVE ops (no partition broadcast).
    # Row 32 (-> M32c): [sum_j v | junk]; col 32 = f * [sum_j k].
    Msb = ping.tile([D + 1, 2 * D], F32R, tag="Msb")
    nc.vector.tensor_scalar_mul(Msb[0:32, 0:32], GG_ps[0:32, 96:128], f10)
    nc.vector.tensor_scalar_mul(Msb[0:32, 32:33], GG_ps[0:32, 32:33], f10)
    nc.scalar.copy(Msb[32:33, 0:32], GG_ps[32:33, 96:128])
    nc.gpsimd.memset(Msb[32:33, 32:33].bitcast(F32), float(L))
    w = 1
    while 32 + w < 2 * D:
        cw = min(w, 2 * D - 32 - w)
        nc.vector.tensor_copy(Msb[:, 32 + w:32 + w + cw],
                              Msb[:, 32:32 + cw])
        w += cw
    m32t_ps = psMG.tile([D + 1, 1], F32, tag="mg")
    nc.tensor.transpose(m32t_ps, Msb[32:33, 0:33].bitcast(F32),
                        identF[32:33, 32:33])
    M32c = ping.tile([D + 1, 1], F32, tag="M32c")
    nc.vector.tensor_copy(M32c, m32t_ps)
    G_ps = psMG.tile([128, 2, 2 * D], F32, tag="mg")
    for cc in range(2):
        nc.tensor.matmul(G_ps[:, cc, :], w2_sb[:, cc, :],
                         Msb[0:32, :], start=True, stop=True)
    Gsb = ping.tile([128, 2, 2 * D], BF16, tag="Gsb")
    nc.vector.tensor_scalar_mul(Gsb, G_ps, 10.0)
    if upto <= 2:
        return

    # ---- T = X^T G + ones*M'[32]; normalize; transpose to OTn --------
    # OTn[u, jb, dd] = O[dd, jb*128+u]  (bf16)
    OTn_sb = ping.tile([128, NJ, D], BF16, tag="OTn")
    TZfull = (ping.tile([D, L], BF16, tag="TZf", name="TZfull")
              if xbar else None)
    for tq in range(8):
        T_ps = psS.tile([2 * D, 512], F32, tag="s")
        for cc in range(2):
            nc.tensor.matmul(T_ps, Gsb[:, cc, :],
                             x_sb[:, cc, tq * 512:(tq + 1) * 512],
                             start=(cc == 0), stop=(cc == 1))
        rzt = work.tile([D, 512], F32, tag="rz", bufs=3)
        _act_recip(nc, rzt, T_ps[32:64, :], float(L))
        if xbar:
            TZ = TZfull[:, tq * 512:(tq + 1) * 512]
        else:
            TZ = work.tile([D, 512], BF16, tag="TZ", bufs=3)
        nc.vector.scalar_tensor_tensor(TZ, T_ps[0:32, :], M32c[0:32],
                                       rzt, mybir.AluOpType.add,
                                       mybir.AluOpType.mult)
        if upto <= 3:
            continue
        if not xbar:
            otn_ps = psKV.tile([128, 4, D], F32, tag="kvt")
            for r in range(4):
                nc.tensor.matmul(otn_ps[:, r, :],
                                 TZ[:, r * 128:(r + 1) * 128],
                                 identB[0:32, 0:32], start=True, stop=True)
            cp_eng[tq % 2](OTn_sb[:, 4 * tq:4 * tq + 4, :], otn_ps)
    if xbar and upto > 3:
        nc.sync.dma_start_transpose(OTn_sb[:], TZfull[:])
    if upto <= 4:
        return
    if DEBUG_DUMP:
        dbg = P["dbg_d"]
        nc.sync.dma_start(dbg[0:32, :], TZfull)
        nc.gpsimd.dma_start(dbg[32:160, 0:NJ * D],
                            OTn_sb.rearrange("p j c -> p (j c)"))

    # ---- R[r', dd, u] via per-dd transposes of OTn -------------------
    R_sb = ping.tile([D, D, 128], BF16, tag="R")
    for ddq in range(8):
        r4_ps = psKV.tile([D, 4, 128], F32, tag="kvt")
        for s in range(4):
            dd = 4 * ddq + s
            nc.tensor.matmul(r4_ps[:, s, :], OTn_sb[:, :, dd],
                             identB, start=True, stop=True)
        cp_eng[ddq % 2](R_sb[:, 4 * ddq:4 * ddq + 4, :], r4_ps)
    if upto <= 5:
        return
    if DEBUG_DUMP:
        nc.sync.dma_start(P["dbg_d"][160:192, :],
                          R_sb.rearrange("p d u -> p (d u)"))

    # ---- final projection: y = wo^T.T @ R ----------------------------
    i = 0
    for mc in range(2):
        for ng in range(2):
            y_sb = work.tile([128, 4, 512], BF16, tag="ysb", bufs=2)
            for sub in range(4):
                ncq = ng * 4 + sub
                y_ps = psS.tile([128, 512], F32, tag="s")
                nc.tensor.matmul(
                    y_ps, wob_sb[:, mc, :],
                    R_sb[:, :, ncq * 16:(ncq + 1) * 16].rearrange(
                        "r d u -> r u d"),
                    start=True, stop=True)
                cp_eng[i % 2](y_sb[:, sub, :], y_ps)
                i += 1
            (nc.sync if (mc + ng) % 2 == 0 else nc.gpsimd).dma_start(
                y_d[mc * 128:(mc + 1) * 128,
                    ng * 2048:(ng + 1) * 2048], y_sb)


def _build_program(repeat=1, upto=99, xbar=False):
    key = ("nc", repeat, upto, xbar)
    names_key = ("names", repeat, upto, xbar)
    if key in _CACHE:
        return _CACHE[key], _CACHE[names_key]
    nc = bacc.Bacc("TRN2", target_bir_lowering=False, debug=False,
                   enable_asserts=False, num_devices=8)
    x_d = nc.dram_tensor("x", (C, L), BF16, kind="ExternalInput").ap()
    wkvm_d = nc.dram_tensor("wkvm", (128, 2, 128), BF16,
                            kind="ExternalInput").ap()
    wqg_d = nc.dram_tensor("wqg", (D, 2, 128), F32R,
                           kind="ExternalInput").ap()
    wob_d = nc.dram_tensor("wob", (D, 2, 128), BF16,
                           kind="ExternalInput").ap()
    y_d = nc.dram_tensor("y", (C, L), BF16, kind="ExternalOutput").ap()
    dbg_d = (nc.dram_tensor("dbg", (192, L), BF16,
                            kind="ExternalOutput").ap()
             if DEBUG_DUMP else None)
    from contextlib import ExitStack
    with tile.TileContext(nc) as tc, ExitStack() as ctx:
        P = {
            "cst": ctx.enter_context(tc.tile_pool(name="cst", bufs=1)),
            "ping": ctx.enter_context(tc.tile_pool(name="ping", bufs=2)),
            "work": ctx.enter_context(tc.tile_pool(name="work", bufs=2)),
            "psKV": ctx.enter_context(
                tc.tile_pool(name="psKV", bufs=2, space="PSUM")),
            "psS": ctx.enter_context(
                tc.tile_pool(name="psS", bufs=3, space="PSUM")),
            "psMG": ctx.enter_context(
                tc.tile_pool(name="psMG", bufs=1, space="PSUM")),
        }
        P["dbg_d"] = dbg_d
        _setup(tc, P)
        if repeat == 1:
            _emit(tc, P, y_d, x_d, wkvm_d, wqg_d, wob_d, upto, xbar)
        else:
            with tc.For_i(0, repeat // 2, 1):
                _emit(tc, P, y_d, x_d, wkvm_d, wqg_d, wob_d, upto, xbar)
                _emit(tc, P, y_d, x_d, wkvm_d, wqg_d, wob_d, upto, xbar)
    nc.compile()
    names = dict(x=x_d.name, wkvm=wkvm_d.name, wqg=wqg_d.name,
                 wob=wob_d.name, y=y_d.name)
    _CACHE[key] = nc
    _CACHE[names_key] = names
    return nc, names


def _in_maps(x, w_qkv, w_out, names):
    import ml_dtypes
    maps = []
    for core in range(8):
        b, h = divmod(core, H)
        wq = w_qkv[h * D:(h + 1) * D]                  # [32, 256]
        wk = w_qkv[128 + h * D:128 + (h + 1) * D]
        wv = w_qkv[256 + h * D:256 + (h + 1) * D]
        wkvq = np.zeros((128, 256), np.float32)
        wkvq[0:32], wkvq[64:96], wkvq[96:128] = wk, wq, wv
        wkvm = np.ascontiguousarray(
            wkvq.T.reshape(2, 128, 128).transpose(1, 0, 2))
        wqg = np.ascontiguousarray(wq.reshape(D, 2, 128))
        wob = np.ascontiguousarray(
            w_out[:, h * D:(h + 1) * D].T.reshape(D, 2, 128))
        maps.append({
            names["x"]: np.ascontiguousarray(x[b]).astype(ml_dtypes.bfloat16),
            names["wkvm"]: wkvm.astype(ml_dtypes.bfloat16),
            names["wqg"]: wqg,
            names["wob"]: wob.astype(ml_dtypes.bfloat16),
        })
    return maps


def run(x, w_qkv, w_out, b_out, **spmd_kwargs):
    """Build+run; returns (y_full, BassKernelResults)."""
    x = np.asarray(x, np.float32)
    w_qkv = np.asarray(w_qkv, np.float32)
    w_out = np.asarray(w_out, np.float32)
    b_out = np.asarray(b_out, np.float32)
    repeat = spmd_kwargs.pop("repeat", 1)
    upto = spmd_kwargs.pop("upto", 99)
    xbar = spmd_kwargs.pop("xbar", True)
    nc, names = _build_program(repeat, upto, xbar)
    res = bass_utils.run_bass_kernel_spmd(
        nc, _in_maps(x, w_qkv, w_out, names), core_ids=list(range(8)),
        **spmd_kwargs)
    y = np.zeros((B, C, L), np.float32)
    for core in range(8):
        y[core // H] += np.asarray(res.results[core][names["y"]],
                                   dtype=np.float32)
    y += b_out[None, :, None]
    return y, res


def kernel(x, w_qkv, w_out, b_out):
    y, _ = run(x, w_qkv, w_out, b_out)
    return y


# revision 52
# speedup vs baseline: 1.0707x; 1.0456x over previous
"""Trainium2 Bass kernel for nn_Attention_18399639896530.

Reference computation (b=2, c=256, l=4096, heads=4, dim_head=32):
  qkv   = w_qkv @ x[b]                  (pointwise conv == channel matmul)
  q,k,v -> (b, h, d, l);  q,k L2-normalized over the *sequence* axis l
  sim   = 10 * q^T k    (per b,h: (l, l));  attn = softmax(sim, -1)
  out   = attn @ v^T -> (b, h, l, d);  y = w_out @ scrambled-reshape + b_out

Key numerical fact: because q,k are normalized along the SEQUENCE axis,
|sim| <= ~0.11 on these inputs, so exp(sim) = 1 + sim to 1.4e-4 relative
accuracy (the gate is 2e-2).  The softmax therefore collapses to LINEAR
attention computed through two tiny matrices:

  kT1 = [K^T | 1]  (4096 x 33),  vT1 = [V^T | 1]
  M'  = kT1^T vT1                       (33 x 33; row 32 = [sum_j v_j | L])
  T   = X^T (Wq^T diag(10 rq rk) M'[0:32]) + 1 * M'[32]     (L x 33)
        --- per-column i: T[i, 0:32] = sum_j e_ij v_j,  T[i,32] = Z_i
  O   = T[:, 0:32] / T[:, 32]  ->  scrambled reshape -> y = wo^T.T @ R

The scrambled reshape needs R[r', (u, dd)] with r' = i//128 on partitions.
Instead of partition-flattening DMAs (per-partition SBUF bandwidth bound,
~6us each), R is built with two rounds of PE transposes:
  TZ [dd, i]  --32x [32,128] transposes-->  OTn [u, jb, dd]
  OTn[:, :, dd]  --32x [128,32-strided] transposes-->  R [r', dd, u]
P1 computes kvT directly (x blocks stationary) so kT/vT/qT land in
transposed layout straight from the projection matmul - no kvN staging.
Sharding: 8 cores == 8 (b, h) pairs; host sums the 4 per-head partials per
batch and adds b_out.
"""

import os
import sys
import numpy as np

try:
    import concourse  # noqa: F401
except ImportError:  # pragma: no cover
    sys.path.insert(0, "/opt/trn_rl_repo")

import concourse.bass as bass  # noqa: E402
import concourse.tile as tile  # noqa: E402
from concourse import bacc, mybir  # noqa: E402
from concourse import bass_utils  # noqa: E402
from concourse.masks import make_identity  # noqa: E402

B, C, L = 2, 256, 4096
H, D = 4, 32
NJ = L // 128       # 32 j-blocks
F32 = mybir.dt.float32
F32R = mybir.dt.float32r
BF16 = mybir.dt.bfloat16

_CACHE = {}
DEBUG_DUMP = False


def _act_recip(nc, out, in_, bias):
    """out = 1/(in_ + bias) on the Activation engine (single-pass table
    op).  bass's activation() refuses Reciprocal wholesale; the achievable
    accuracy (~1e-5 relative here, denominators ~4096) is far inside this
    kernel's 2e-2 budget."""
    imm = lambda v: mybir.ImmediateValue(dtype=mybir.dt.float32, value=v)
    return nc.scalar.add_instruction(
        mybir.InstActivation(
            name=nc.get_next_instruction_name(),
            func=mybir.ActivationFunctionType.Reciprocal,
            ins=[nc.scalar.lower_ap(in_), imm(bias), imm(1.0), imm(0.0)],
            outs=[nc.scalar.lower_ap(out)],
        ))


def _setup(tc, P):
    """Compile-time constants, emitted once before the repeat loop."""
    nc = tc.nc
    cst = P["cst"]
    identF = cst.tile([128, 128], F32)
    make_identity(nc, identF)
    identB = cst.tile([128, 128], BF16)
    nc.vector.tensor_copy(identB, identF)
    P["identF"] = identF
    P["identB"] = identB


def _emit(tc, P, y_d, x_d, wkvm_d, wqg_d, wob_d, upto=99, xbar=False):
    nc = tc.nc
    ping, work = P["ping"], P["work"]
    psKV, psS, psMG = P["psKV"], P["psS"], P["psMG"]
    identF, identB = P["identF"], P["identB"]
    ident = identF[0:D + 1, 0:D + 1]

    # ---- load inputs (small weights first) ---------------------------
    wkq_sb = ping.tile([128, 2, 3 * D], BF16, tag="wkq")  # [c, cc, wk|wq|wv]
    nc.sync.dma_start(wkq_sb, wkvm_d)
    w2_sb = ping.tile([D, 2, 128], F32R, tag="w2")        # [a, cc, wq rows]
    nc.gpsimd.dma_start(w2_sb, wqg_d)
    wob_sb = ping.tile([D, 2, 128], BF16, tag="wob")      # [r', mc, wo rows]
    nc.gpsimd.dma_start(wob_sb, wob_d)
    x_sb = ping.tile([128, 2, L], BF16, tag="x")          # [c%128, c//128, l]
    xr = x_d.rearrange("(cc p) l -> p cc l", p=128)
    for lq in range(4):
        (nc.sync if lq % 2 == 0 else nc.gpsimd).dma_start(
            x_sb[:, :, lq * 1024:(lq + 1) * 1024],
            xr[:, :, lq * 1024:(lq + 1) * 1024])

    # kvT layout [j%128, jb, 99]: [kT |1| qT |1| vT |1] (3 x 33 groups).
    # Gram rhs = groups 0,2 = [kT1 | vT1] via group-strided AP:
    #   MG[0:33, 0:33] = kT1 gram (k norms);  MG[0:33, 33:66] = M'raw
    kvT_sb = ping.tile([128, NJ, 99], BF16, tag="kvT")
    nc.gpsimd.memset(kvT_sb.rearrange(
        "p j (g c) -> p j g c", g=3)[:, :, :, 32:33], 1.0)

    # ---- P1: kvT direct (x blocks stationary) + gram -----------------
    # out[j, r] = sum_c x[c, jb*128+j] wkqv[r, c]  ->  kvT block [128, 96]
    MG_ps = psMG.tile([D + 1, 66], F32, tag="mg")
    qg_ps = psMG.tile([D, D], F32, tag="qg")
    cp_eng = [nc.vector.tensor_copy, nc.scalar.copy]

    def _gram(lq):
        for t in range(4):
            jb = 4 * lq + t
            kv1 = kvT_sb[:, jb, :].rearrange(
                "p (g c) -> p g c", g=3)[:, 0::2, :]
            nc.tensor.matmul(MG_ps, kvT_sb[:, jb, 0:33], kv1,
                             start=(jb == 0), stop=(jb == NJ - 1),
                             skip_group_check=True)
            nc.tensor.matmul(qg_ps, kvT_sb[:, jb, 33:65],
                             kvT_sb[:, jb, 33:65],
                             start=(jb == 0), stop=(jb == NJ - 1),
                             skip_group_check=True)

    for lq in range(8):
        for t2 in range(2):
            jb = 4 * lq + 2 * t2
            kvt_ps = psKV.tile([128, 2, 3 * D], F32, tag="kvt")
            for half in range(2):
                for cc in range(2):
                    nc.tensor.matmul(kvt_ps[:, half, :],
                                     x_sb[:, cc,
                                          (jb + half) * 128:
                                          (jb + half + 1) * 128],
                                     wkq_sb[:, cc, :],
                                     start=(cc == 0), stop=(cc == 1),
                                     skip_group_check=True)
            dst = kvT_sb[:, jb:jb + 2, :].rearrange(
                "p j (g c) -> p j g c", g=3)[:, :, :, 0:32]
            cp_eng[jb % 2](dst, kvt_ps.rearrange(
                "p j (g c) -> p j g c", g=3))
        if lq > 0:
            _gram(lq - 1)
    _gram(7)
    if upto <= 1:
        return

    # ---- fold both norms + SCALE into 10/(||q_a|| ||k_a||) -----------
    gd2 = ping.tile([D, D], F32, tag="gd2")
    nc.vector.tensor_mul(gd2, qg_ps, identF[0:D, 0:D])
    nqs = ping.tile([D, 1], F32, tag="nqs")
    nc.vector.tensor_reduce(nqs, gd2, axis=mybir.AxisListType.X,
                            op=mybir.AluOpType.add)
    gd = ping.tile([D + 1, D + 1], F32, tag="gd")
    nc.vector.tensor_mul(gd, MG_ps[:, 0:33], ident)
    nks = ping.tile([D + 1, 1], F32, tag="nks")
    nc.vector.tensor_reduce(nks, gd, axis=mybir.AxisListType.X,
                            op=mybir.AluOpType.add)
    m = ping.tile([D, 1], F32, tag="m")
    nc.vector.tensor_mul(m, nqs, nks[0:32])
    sqm = ping.tile([D, 1], F32, tag="sqm")
    nc.scalar.activation(sqm, m, mybir.ActivationFunctionType.Sqrt)
    f10 = ping.tile([D, 1], F32, tag="f10")
    nc.vector.reciprocal(f10, sqm)

    # ---- Msb = diag([f | 1]) M'raw, Z-col replicated to 33:64 --------
    # (x10 folded into Gsb).  T rows 32:64 all carry Z so the epilogue
    # reciprocal+mul are full-width DVE ops (no partition broadcast).
    Msb = ping.tile([D + 1, 2 * D], F32R, tag="Msb")
    nc.vector.tensor_scalar_mul(Msb[0:32, 0:33], MG_ps[0:32, 33:66], f10)
    nc.scalar.copy(Msb[32:33, 0:33], MG_ps[32:33, 33:66])
    w = 1
    while 32 + w < 2 * D:
        cw = min(w, 2 * D - 32 - w)
        nc.vector.tensor_copy(Msb[:, 32 + w:32 + w + cw],
                              Msb[:, 32:32 + cw])
        w += cw
    m32t_ps = psMG.tile([D + 1, 1], F32, tag="mg")
    nc.tensor.transpose(m32t_ps, Msb[32:33, 0:33].bitcast(F32),
                        identF[32:33, 32:33])
    M32c = ping.tile([D + 1, 1], F32, tag="M32c")
    nc.vector.tensor_copy(M32c, m32t_ps)
    G_ps = psMG.tile([128, 2, 2 * D], F32, tag="mg")
    for cc in range(2):
        nc.tensor.matmul(G_ps[:, cc, :], w2_sb[:, cc, :],
                         Msb[0:32, :], start=True, stop=True)
    Gsb = ping.tile([128, 2, 2 * D], BF16, tag="Gsb")
    nc.vector.tensor_scalar_mul(Gsb, G_ps, 10.0)
    if upto <= 2:
        return

    # ---- T = X^T G + ones*M'[32]; normalize; transpose to OTn --------
    # OTn[u, jb, dd] = O[dd, jb*128+u]  (bf16)
    OTn_sb = ping.tile([128, NJ, D], BF16, tag="OTn")
    TZfull = (ping.tile([D, L], BF16, tag="TZf", name="TZfull")
              if xbar else None)
    for tq in range(8):
        T_ps = psS.tile([2 * D, 512], F32, tag="s")
        for cc in range(2):
            nc.tensor.matmul(T_ps, Gsb[:, cc, :],
                             x_sb[:, cc, tq * 512:(tq + 1) * 512],
                             start=(cc == 0), stop=(cc == 1))
        rzt = work.tile([D, 512], F32, tag="rz", bufs=3)
        _act_recip(nc, rzt, T_ps[32:64, :], float(L))
        if xbar:
            TZ = TZfull[:, tq * 512:(tq + 1) * 512]
        else:
            TZ = work.tile([D, 512], BF16, tag="TZ", bufs=3)
        nc.vector.scalar_tensor_tensor(TZ, T_ps[0:32, :], M32c[0:32],
                                       rzt, mybir.AluOpType.add,
                                       mybir.AluOpType.mult)
        if upto <= 3:
            continue
        if not xbar:
            otn_ps = psKV.tile([128, 4, D], F32, tag="kvt")
            for r in range(4):
                nc.tensor.matmul(otn_ps[:, r, :],
                                 TZ[:, r * 128:(r + 1) * 128],
                                 identB[0:32, 0:32], start=True, stop=True)
            cp_eng[tq % 2](OTn_sb[:, 4 * tq:4 * tq + 4, :], otn_ps)
    if xbar and upto > 3:
        nc.sync.dma_start_transpose(OTn_sb[:], TZfull[:])
    if upto <= 4:
        return
    if DEBUG_DUMP:
        dbg = P["dbg_d"]
        nc.sync.dma_start(dbg[0:32, :], TZfull)
        nc.gpsimd.dma_start(dbg[32:160, 0:NJ * D],
                            OTn_sb.rearrange("p j c -> p (j c)"))

    # ---- R[r', dd, u] via per-dd transposes of OTn -------------------
    R_sb = ping.tile([D, D, 128], BF16, tag="R")
    for ddq in range(8):
        r4_ps = psKV.tile([D, 4, 128], F32, tag="kvt")
        for s in range(4):
            dd = 4 * ddq + s
            nc.tensor.matmul(r4_ps[:, s, :], OTn_sb[:, :, dd],
                             identB, start=True, stop=True)
        cp_eng[ddq % 2](R_sb[:, 4 * ddq:4 * ddq + 4, :], r4_ps)
    if upto <= 5:
        return
    if DEBUG_DUMP:
        nc.sync.dma_start(P["dbg_d"][160:192, :],
                          R_sb.rearrange("p d u -> p (d u)"))

    # ---- final projection: y = wo^T.T @ R ----------------------------
    i = 0
    for mc in range(2):
        for ng in range(2):
            y_sb = work.tile([128, 4, 512], BF16, tag="ysb", bufs=2)
            for sub in range(4):
                ncq = ng * 4 + sub
                y_ps = psS.tile([128, 512], F32, tag="s")
                nc.tensor.matmul(
                    y_ps, wob_sb[:, mc, :],
                    R_sb[:, :, ncq * 16:(ncq + 1) * 16].rearrange(
                        "r d u -> r u d"),
                    start=True, stop=True)
                cp_eng[i % 2](y_sb[:, sub, :], y_ps)
                i += 1
            (nc.sync if (mc + ng) % 2 == 0 else nc.gpsimd).dma_start(
                y_d[mc * 128:(mc + 1) * 128,
                    ng * 2048:(ng + 1) * 2048], y_sb)


def _build_program(repeat=1, upto=99, xbar=False):
    key = ("nc", repeat, upto, xbar)
    names_key = ("names", repeat, upto, xbar)
    if key in _CACHE:
        return _CACHE[key], _CACHE[names_key]
    nc = bacc.Bacc("TRN2", target_bir_lowering=False, debug=False,
                   enable_asserts=False, num_devices=8)
    x_d = nc.dram_tensor("x", (C, L), BF16, kind="ExternalInput").ap()
    wkvm_d = nc.dram_tensor("wkvm", (128, 2, 3 * D), BF16,
                            kind="ExternalInput").ap()
    wqg_d = nc.dram_tensor("wqg", (D, 2, 128), F32R,
                           kind="ExternalInput").ap()
    wob_d = nc.dram_tensor("wob", (D, 2, 128), BF16,
                           kind="ExternalInput").ap()
    y_d = nc.dram_tensor("y", (C, L), BF16, kind="ExternalOutput").ap()
    dbg_d = (nc.dram_tensor("dbg", (192, L), BF16,
                            kind="ExternalOutput").ap()
             if DEBUG_DUMP else None)
    from contextlib import ExitStack
    with tile.TileContext(nc) as tc, ExitStack() as ctx:
        P = {
            "cst": ctx.enter_context(tc.tile_pool(name="cst", bufs=1)),
            "ping": ctx.enter_context(tc.tile_pool(name="ping", bufs=2)),
            "work": ctx.enter_context(tc.tile_pool(name="work", bufs=2)),
            "psKV": ctx.enter_context(
                tc.tile_pool(name="psKV", bufs=2, space="PSUM")),
            "psS": ctx.enter_context(
                tc.tile_pool(name="psS", bufs=3, space="PSUM")),
            "psMG": ctx.enter_context(
                tc.tile_pool(name="psMG", bufs=1, space="PSUM")),
        }
        P["dbg_d"] = dbg_d
        _setup(tc, P)
        if repeat == 1:
            _emit(tc, P, y_d, x_d, wkvm_d, wqg_d, wob_d, upto, xbar)
        else:
            with tc.For_i(0, repeat // 2, 1):
                _emit(tc, P, y_d, x_d, wkvm_d, wqg_d, wob_d, upto, xbar)
                _emit(tc, P, y_d, x_d, wkvm_d, wqg_d, wob_d, upto, xbar)
    nc.compile()
    names = dict(x=x_d.name, wkvm=wkvm_d.name, wqg=wqg_d.name,
                 wob=wob_d.name, y=y_d.name)
    _CACHE[key] = nc
    _CACHE[names_key] = names
    return nc, names


def _in_maps(x, w_qkv, w_out, names):
    import ml_dtypes
    maps = []
    for core in range(8):
        b, h = divmod(core, H)
        wq = w_qkv[h * D:(h + 1) * D]                  # [32, 256]
        wk = w_qkv[128 + h * D:128 + (h + 1) * D]
        wv = w_qkv[256 + h * D:256 + (h + 1) * D]
        wkvq = np.concatenate([wk, wq, wv], 0)         # [96, 256]
        wkvm = np.ascontiguousarray(
            wkvq.T.reshape(2, 128, 3 * D).transpose(1, 0, 2))
        wqg = np.ascontiguousarray(wq.reshape(D, 2, 128))
        wob = np.ascontiguousarray(
            w_out[:, h * D:(h + 1) * D].T.reshape(D, 2, 128))
        maps.append({
            names["x"]: np.ascontiguousarray(x[b]).astype(ml_dtypes.bfloat16),
            names["wkvm"]: wkvm.astype(ml_dtypes.bfloat16),
            names["wqg"]: wqg,
            names["wob"]: wob.astype(ml_dtypes.bfloat16),
        })
    return maps


def run(x, w_qkv, w_out, b_out, **spmd_kwargs):
    """Build+run; returns (y_full, BassKernelResults)."""
    x = np.asarray(x, np.float32)
    w_qkv = np.asarray(w_qkv, np.float32)
    w_out = np.asarray(w_out, np.float32)
    b_out = np.asarray(b_out, np.float32)
    repeat = spmd_kwargs.pop("repeat", 1)
    upto = spmd_kwargs.pop("upto", 99)
    xbar = spmd_kwargs.pop("xbar", True)
    nc, names = _build_program(repeat, upto, xbar)
    res = bass_utils.run_bass_kernel_spmd(
        nc, _in_maps(x, w_qkv, w_out, names), core_ids=list(range(8)),
        **spmd_kwargs)
    y = np.zeros((B, C, L), np.float32)
    for core in range(8):
        y[core // H] += np.asarray(res.results[core][names["y"]],
                                   dtype=np.float32)
    y += b_out[None, :, None]
    return y, res


def kernel(x, w_qkv, w_out, b_out):
    y, _ = run(x, w_qkv, w_out, b_out)
    return y
